# revision 16
# baseline (speedup 1.0000x reference)
"""Multi-head attention (B=4, S=2048, D=512, H=8, Dh=64) on 8 Trainium2 cores.

Sharding: core i handles batch b = i//2 and head-group hg = i%2 (4 heads each).
Data parallel on B, tensor parallel on heads / QKV projection columns.

Per-core kernel (all matmuls fp32r, fp32 PSUM accumulate):
  inputs : xT [512, 2048]  (x[b] transposed, host-side)
           w  [512, 768]   (QKV projection columns for this head group)
  output : outT [256, 2048] (row h*64+d, col q) -- host transposes/interleaves.

  phase 1: Q^T, K^T = w_q/w_k^T-as-lhsT @ xT   -> [64*4, 2048] per-head rows
           V natural = xT-as-lhsT @ w_v        -> [2048, 4, 64] (+ ones col)
  phase 2: per (head, q-chunk of 1024), stream k-chunks of 128:
           dots^T[k, q] = K^T-chunk^T @ Q^T    (PSUM [128, 1024])
           P^T = exp(0.125 * dots^T)           (ScalarE, PSUM->SBUF, f32r)
           acc[65, q] += [V | 1]^T @ P^T       (row 64 = softmax denominator)
           out^T = acc[0:64] * bcast(1/acc[64])
"""
import sys

sys.path.insert(0, "/opt/trn_rl_repo")

import numpy as np

B, S, D = 4, 2048, 512
H_TOT, H, DH = 8, 4, 64  # heads total / per core
DC = D // 128  # 4 contraction chunks
KC = S // 128  # 16 k chunks
NQ = 4  # q chunks of QCH
QCH = 512
NJ = QCH // 512  # 512-wide matmul slices per q chunk
SCALE = DH ** -0.5

_CACHE = {}


def _build():
    import concourse.tile as tile
    from concourse import bacc, mybir

    F32 = mybir.dt.float32
    F32R = mybir.dt.float32r
    BF16 = mybir.dt.bfloat16
    EXP = mybir.ActivationFunctionType.Exp

    nc = bacc.Bacc("TRN2", target_bir_lowering=False, debug=False, num_devices=8)

    xT_d = nc.dram_tensor("xT", [D, S], F32, kind="ExternalInput")
    w_d = nc.dram_tensor("w", [D, 3 * H * DH], F32, kind="ExternalInput")
    out_d = nc.dram_tensor("outT", [H * DH, S], F32, kind="ExternalOutput")

    with tile.TileContext(nc) as tc:
        with tc.tile_pool(name="resident", bufs=1) as res:
            xT = res.tile([128, DC, S], F32R)
            w = res.tile([128, DC, 3 * H * DH], F32R)
            qT = res.tile([128, 2, S], F32R)
            kT = res.tile([128, 2, S], F32R)
            v = res.tile([128, KC, H, DH + 1], BF16)
            ones = res.tile([65, 64], F32R)
            nc.vector.memset(ones[64:65, :].bitcast(F32), 1.0)

            # preload the exp table set early (hides the ~2.7us ACT_TABLE_LOAD
            # under the initial DMAs instead of stalling PE mid-kernel)
            pre_in = res.tile([1, 2], F32)
            pre_out = res.tile([1, 2], F32)
            nc.vector.memset(pre_in[:], 0.0)
            nc.scalar.activation(
                pre_out[:], pre_in[:], mybir.ActivationFunctionType.Exp, scale=1.0
            )

            for dc in range(DC):
                nc.sync.dma_start(
                    xT[:, dc, :], xT_d[dc * 128 : (dc + 1) * 128, :].bitcast(F32R)
                )
                nc.sync.dma_start(
                    w[:, dc, :], w_d[dc * 128 : (dc + 1) * 128, :].bitcast(F32R)
                )
            nc.vector.memset(v[:, :, :, DH : DH + 1], 1.0)

            # ---- phase 1: projections ----
            with tc.tile_pool(name="proj_ps", bufs=2, space="PSUM") as proj_ps:
                # Q^T / K^T: lhsT = w columns (4 chunks of 128: q01,q23,k01,k23)
                for cc in range(4):
                    for sc in range(4):
                        ps = proj_ps.tile([128, 512], F32)
                        for dc in range(DC):
                            nc.tensor.matmul(
                                ps[:],
                                w[:, dc, cc * 128 : (cc + 1) * 128],
                                xT[:, dc, sc * 512 : (sc + 1) * 512],
                                start=(dc == 0),
                                stop=(dc == DC - 1),
                            )
                        dst = qT if cc < 2 else kT
                        nc.vector.tensor_copy(
                            dst[:, cc % 2, sc * 512 : (sc + 1) * 512], ps[:]
                        )
                # V natural: lhsT = xT chunks, rhs = w_v columns
                for sc in range(KC):
                    ps = proj_ps.tile([128, 256], F32, tag="vps")
                    for dc in range(DC):
                        nc.tensor.matmul(
                            ps[:],
                            xT[:, dc, sc * 128 : (sc + 1) * 128],
                            w[:, dc, 2 * H * DH : 3 * H * DH],
                            start=(dc == 0),
                            stop=(dc == DC - 1),
                        )
                    nc.vector.tensor_copy(
                        v[:, sc, :, 0:DH],
                        ps[:].rearrange("p (h d) -> p h d", h=H),
                    )

            # ---- phase 2: attention ----
            # Combined two-head dots tile per k-chunk: one FD-1024 exp per kc
            # amortizes the ScalarE per-instruction bubble. Dummy bf16 matmuls
            # (DUMMY_PER_KC) keep TensorE duty high enough that the clock
            # governor holds K=8/8 (2.4 GHz) through the exp-gated phase.
            DUMMY_PER_KC = 2
            with (
                tc.tile_pool(name="dots_ps", bufs=2, space="PSUM") as dots_ps,
                tc.tile_pool(name="oacc_ps", bufs=3, space="PSUM") as oacc_ps,
                tc.tile_pool(name="scr_ps", bufs=1, space="PSUM") as scr_ps,
                tc.tile_pool(name="pt", bufs=3) as pt_pool,
                tc.tile_pool(name="norm", bufs=2) as norm_pool,
                tc.tile_pool(name="outp", bufs=2) as out_pool,
            ):
                scratch = scr_ps.tile([128, 512], F32)
                for hc in range(2):
                    for qc in range(NQ):
                        q0 = qc * QCH
                        oaccs = [
                            oacc_ps.tile(
                                [DH + 1, QCH], F32, tag="oacc", name=f"oacc{_hp}"
                            )
                            for _hp in range(2)
                        ]
                        for kc in range(KC):
                            dots = dots_ps.tile([128, 2, QCH], F32, tag="dots")
                            for hp in range(2):
                                pb = hp * 64
                                nc.tensor.matmul(
                                    dots[:, hp, :],
                                    kT[pb : pb + 64, hc, kc * 128 : (kc + 1) * 128],
                                    qT[pb : pb + 64, hc, q0 : q0 + QCH],
                                    start=True,
                                    stop=True,
                                )
                            pT = pt_pool.tile([128, 2, QCH], BF16, tag="pt")
                            nc.scalar.activation(pT[:], dots[:], EXP, scale=SCALE)
                            for hp in range(2):
                                h = 2 * hc + hp
                                nc.tensor.matmul(
                                    oaccs[hp][:],
                                    v[:, kc, h, :],
                                    pT[:, hp, :],
                                    start=(kc == 0),
                                    stop=(kc == KC - 1),
                                )
                            for du in range(DUMMY_PER_KC):
                                nc.tensor.matmul(
                                    scratch[:],
                                    pT[:, 0, 0:128],
                                    pT[:, 1, :],
                                    start=True,
                                    stop=True,
                                    skip_group_check=True,
                                )
                        for hp in range(2):
                            h = 2 * hc + hp
                            # move acc out of PSUM fast, normalize from SBUF
                            osb = norm_pool.tile([DH + 1, QCH], F32, tag="osb")
                            nc.vector.tensor_copy(osb[:], oaccs[hp][:])
                            # full-tile: custom DVE ops misbehave at nonzero
                            # base partition; rows 0:64 are unused garbage.
                            recf = norm_pool.tile([DH + 1, QCH], F32, tag="recf")
                            nc.vector.reciprocal_approx_fast(recf[:], osb[:])
                            recr = norm_pool.tile([DH + 1, QCH], F32R, tag="recr")
                            with nc.allow_low_precision(
                                reason="f32r recip of softmax denom (~1e-4)"
                            ):
                                nc.vector.tensor_copy(
                                    recr[DH : DH + 1, :], recf[DH : DH + 1, :]
                                )
                            # broadcast recip row to 64 partitions via K=1 matmul
                            rb_ps = dots_ps.tile([128, 2, QCH], F32, tag="dots")
                            nc.tensor.matmul(
                                rb_ps[0:DH, 0, :],
                                ones[64:65, :],
                                recr[DH : DH + 1, :],
                                start=True,
                                stop=True,
                            )
                            rb = norm_pool.tile([DH, QCH], F32, tag="rb")
                            nc.vector.tensor_copy(rb[:], rb_ps[0:DH, 0, :])
                            out_t = out_pool.tile([DH, QCH], F32)
                            nc.vector.tensor_mul(out_t[:], osb[0:DH, :], rb[:])
                            nc.sync.dma_start(
                                out_d[h * DH : (h + 1) * DH, q0 : q0 + QCH],
                                out_t[:],
                            )

    nc.compile()
    return nc


def kernel(x, w_qkv):
    from concourse.bass_utils import run_bass_kernel_spmd

    x = np.asarray(x)
    w_qkv = np.asarray(w_qkv)

    if "nc" not in _CACHE:
        _CACHE["nc"] = _build()
    nc = _CACHE["nc"]

    in_maps = []
    for i in range(8):
        b, hg = i // 2, i % 2
        xT = np.ascontiguousarray(x[b].T, dtype=np.float32)
        cols = slice(hg * H * DH, (hg + 1) * H * DH)
        w_i = np.ascontiguousarray(
            np.concatenate(
                [
                    w_qkv[:, 0 * D :][:, cols],
                    w_qkv[:, 1 * D :][:, cols],
                    w_qkv[:, 2 * D :][:, cols],
                ],
                axis=1,
            ),
            dtype=np.float32,
        )
        in_maps.append({"xT": xT, "w": w_i})
    _CACHE["in_maps"] = in_maps

    res = run_bass_kernel_spmd(nc, in_maps, list(range(8)))

    out = np.empty((B, S, D), dtype=np.float32)
    for i in range(8):
        b, hg = i // 2, i % 2
        out[b, :, hg * H * DH : (hg + 1) * H * DH] = res.results[i]["outT"].T
    return out


# revision 17
# speedup vs baseline: 1.5396x; 1.5396x over previous
"""Multi-head attention (B=4, S=2048, D=512, H=8, Dh=64) on 8 Trainium2 cores.

Sharding: core i handles batch b = i//2 and head-group hg = i%2 (4 heads each).
Data parallel on B, tensor parallel on heads / QKV projection columns.

Per-core kernel (all matmuls fp32r, fp32 PSUM accumulate):
  inputs : xT [512, 2048]  (x[b] transposed, host-side)
           w  [512, 768]   (QKV projection columns for this head group)
  output : outT [256, 2048] (row h*64+d, col q) -- host transposes/interleaves.

  phase 1: Q^T, K^T = w_q/w_k^T-as-lhsT @ xT   -> [64*4, 2048] per-head rows
           V natural = xT-as-lhsT @ w_v        -> [2048, 4, 64] (+ ones col)
  phase 2: per (head, q-chunk of 1024), stream k-chunks of 128:
           dots^T[k, q] = K^T-chunk^T @ Q^T    (PSUM [128, 1024])
           P^T = exp(0.125 * dots^T)           (ScalarE, PSUM->SBUF, f32r)
           acc[65, q] += [V | 1]^T @ P^T       (row 64 = softmax denominator)
           out^T = acc[0:64] * bcast(1/acc[64])
"""
import sys

sys.path.insert(0, "/opt/trn_rl_repo")

import numpy as np

B, S, D = 4, 2048, 512
H_TOT, H, DH = 8, 4, 64  # heads total / per core
DC = D // 128  # 4 contraction chunks
KC = S // 128  # 16 k chunks
NQ = 4  # q chunks of QCH
QCH = 512
NJ = QCH // 512  # 512-wide matmul slices per q chunk
SCALE = DH ** -0.5

_CACHE = {}


def _build():
    import concourse.tile as tile
    from concourse import bacc, mybir

    F32 = mybir.dt.float32
    F32R = mybir.dt.float32r
    BF16 = mybir.dt.bfloat16
    EXP = mybir.ActivationFunctionType.Exp

    nc = bacc.Bacc("TRN2", target_bir_lowering=False, debug=False, num_devices=8)

    xT_d = nc.dram_tensor("xT", [D, S], F32, kind="ExternalInput")
    w_d = nc.dram_tensor("w", [D, 3 * H * DH], F32, kind="ExternalInput")
    out_d = nc.dram_tensor("outT", [H * DH, S], F32, kind="ExternalOutput")

    with tile.TileContext(nc) as tc:
        with tc.tile_pool(name="resident", bufs=1) as res:
            xT = res.tile([128, DC, S], F32R)
            w = res.tile([128, DC, 3 * H * DH], F32R)
            qT = res.tile([128, 2, S], F32R)
            kT = res.tile([128, 2, S], F32R)
            v = res.tile([128, KC, H, DH + 1], BF16)
            ones = res.tile([65, 64], F32R)
            nc.vector.memset(ones[64:65, :].bitcast(F32), 1.0)

            # preload the exp table set early (hides the ~2.7us ACT_TABLE_LOAD
            # under the initial DMAs instead of stalling PE mid-kernel)
            pre_in = res.tile([1, 2], F32)
            pre_out = res.tile([1, 2], F32)
            nc.vector.memset(pre_in[:], 0.0)
            nc.scalar.activation(
                pre_out[:], pre_in[:], mybir.ActivationFunctionType.Exp, scale=1.0
            )

            for dc in range(DC):
                nc.sync.dma_start(
                    xT[:, dc, :], xT_d[dc * 128 : (dc + 1) * 128, :].bitcast(F32R)
                )
                nc.sync.dma_start(
                    w[:, dc, :], w_d[dc * 128 : (dc + 1) * 128, :].bitcast(F32R)
                )
            nc.vector.memset(v[:, :, :, DH : DH + 1], 1.0)

            # ---- phase 1: projections ----
            with tc.tile_pool(name="proj_ps", bufs=2, space="PSUM") as proj_ps:
                # Q^T / K^T: lhsT = w columns (4 chunks of 128: q01,q23,k01,k23)
                for cc in range(4):
                    for sc in range(4):
                        ps = proj_ps.tile([128, 512], F32)
                        for dc in range(DC):
                            nc.tensor.matmul(
                                ps[:],
                                w[:, dc, cc * 128 : (cc + 1) * 128],
                                xT[:, dc, sc * 512 : (sc + 1) * 512],
                                start=(dc == 0),
                                stop=(dc == DC - 1),
                            )
                        dst = qT if cc < 2 else kT
                        nc.vector.tensor_copy(
                            dst[:, cc % 2, sc * 512 : (sc + 1) * 512], ps[:]
                        )
                # V natural: lhsT = xT chunks, rhs = w_v columns
                for sc in range(KC):
                    ps = proj_ps.tile([128, 256], F32, tag="vps")
                    for dc in range(DC):
                        nc.tensor.matmul(
                            ps[:],
                            xT[:, dc, sc * 128 : (sc + 1) * 128],
                            w[:, dc, 2 * H * DH : 3 * H * DH],
                            start=(dc == 0),
                            stop=(dc == DC - 1),
                        )
                    nc.vector.tensor_copy(
                        v[:, sc, :, 0:DH],
                        ps[:].rearrange("p (h d) -> p h d", h=H),
                    )

            # ---- phase 2: attention ----
            # Combined two-head dots tile per k-chunk: one FD-1024 exp per kc
            # amortizes the ScalarE per-instruction bubble. Dummy bf16 matmuls
            # (DUMMY_PER_KC) keep TensorE duty high enough that the clock
            # governor holds K=8/8 (2.4 GHz) through the exp-gated phase.
            DUMMY_PER_KC = 1
            with (
                tc.tile_pool(name="dots_ps", bufs=2, space="PSUM") as dots_ps,
                tc.tile_pool(name="oacc_ps", bufs=4, space="PSUM") as oacc_ps,
                tc.tile_pool(name="pt", bufs=3) as pt_pool,
                tc.tile_pool(name="norm", bufs=2) as norm_pool,
                tc.tile_pool(name="outp", bufs=2) as out_pool,
            ):
                for hc in range(2):
                    for qc in range(NQ):
                        q0 = qc * QCH
                        oaccs = [
                            oacc_ps.tile(
                                [DH + 1, QCH], F32, tag="oacc", name=f"oacc{_hp}"
                            )
                            for _hp in range(2)
                        ]
                        for kc in range(KC):
                            dots = dots_ps.tile([128, 2, QCH], F32, tag="dots")
                            for hp in range(2):
                                pb = hp * 64
                                nc.tensor.matmul(
                                    dots[:, hp, :],
                                    kT[pb : pb + 64, hc, kc * 128 : (kc + 1) * 128],
                                    qT[pb : pb + 64, hc, q0 : q0 + QCH],
                                    start=True,
                                    stop=True,
                                )
                            pT = pt_pool.tile([128, 2, QCH], BF16, tag="pt")
                            nc.scalar.activation(pT[:], dots[:], EXP, scale=SCALE)
                            for hp in range(2):
                                h = 2 * hc + hp
                                nc.tensor.matmul(
                                    oaccs[hp][:],
                                    v[:, kc, h, :],
                                    pT[:, hp, :],
                                    start=(kc == 0),
                                    stop=(kc == KC - 1),
                                )
                            for du in range(DUMMY_PER_KC):
                                dummy = oacc_ps.tile(
                                    [DH + 1, QCH], F32, tag="oacc", name="dummy"
                                )
                                nc.tensor.matmul(
                                    dummy[:],
                                    pT[:, 0, 0 : DH + 1],
                                    pT[:, 1, :],
                                    start=True,
                                    stop=True,
                                    skip_group_check=True,
                                )
                        osbs = []
                        for hp in range(2):
                            osb = norm_pool.tile(
                                [DH + 1, QCH], F32, tag="osb", name=f"osb{hp}"
                            )
                            nc.vector.tensor_copy(osb[:], oaccs[hp][:])
                            osbs.append(osb)
                        for hp in range(2):
                            h = 2 * hc + hp
                            osb = osbs[hp]
                            # full-tile: custom DVE ops misbehave at nonzero
                            # base partition; rows 0:64 are unused garbage.
                            recf = norm_pool.tile([DH + 1, QCH], F32, tag="recf")
                            nc.vector.reciprocal_approx_fast(recf[:], osb[:])
                            recr = norm_pool.tile([DH + 1, QCH], F32R, tag="recr")
                            with nc.allow_low_precision(
                                reason="f32r recip of softmax denom (~1e-4)"
                            ):
                                nc.vector.tensor_copy(
                                    recr[DH : DH + 1, :], recf[DH : DH + 1, :]
                                )
                            # broadcast recip row to 64 partitions via K=1 matmul
                            rb_ps = oacc_ps.tile(
                                [DH + 1, QCH], F32, tag="oacc", name="rbps"
                            )
                            nc.tensor.matmul(
                                rb_ps[0:DH, :],
                                ones[64:65, :],
                                recr[DH : DH + 1, :],
                                start=True,
                                stop=True,
                            )
                            rb = norm_pool.tile([DH, QCH], F32, tag="rb")
                            nc.vector.tensor_copy(rb[:], rb_ps[0:DH, :])
                            out_t = out_pool.tile([DH, QCH], F32)
                            nc.vector.tensor_mul(out_t[:], osb[0:DH, :], rb[:])
                            nc.sync.dma_start(
                                out_d[h * DH : (h + 1) * DH, q0 : q0 + QCH],
                                out_t[:],
                            )

    nc.compile()
    return nc


def kernel(x, w_qkv):
    from concourse.bass_utils import run_bass_kernel_spmd

    x = np.asarray(x)
    w_qkv = np.asarray(w_qkv)

    if "nc" not in _CACHE:
        _CACHE["nc"] = _build()
    nc = _CACHE["nc"]

    in_maps = []
    for i in range(8):
        b, hg = i // 2, i % 2
        xT = np.ascontiguousarray(x[b].T, dtype=np.float32)
        cols = slice(hg * H * DH, (hg + 1) * H * DH)
        w_i = np.ascontiguousarray(
            np.concatenate(
                [
                    w_qkv[:, 0 * D :][:, cols],
                    w_qkv[:, 1 * D :][:, cols],
                    w_qkv[:, 2 * D :][:, cols],
                ],
                axis=1,
            ),
            dtype=np.float32,
        )
        in_maps.append({"xT": xT, "w": w_i})
    _CACHE["in_maps"] = in_maps

    res = run_bass_kernel_spmd(nc, in_maps, list(range(8)))

    out = np.empty((B, S, D), dtype=np.float32)
    for i in range(8):
        b, hg = i // 2, i % 2
        out[b, :, hg * H * DH : (hg + 1) * H * DH] = res.results[i]["outT"].T
    return out


# revision 18
# speedup vs baseline: 1.6102x; 1.0458x over previous
"""Multi-head attention (B=4, S=2048, D=512, H=8, Dh=64) on 8 Trainium2 cores.

Sharding: core i handles batch b = i//2 and head-group hg = i%2 (4 heads each).
Data parallel on B, tensor parallel on heads / QKV projection columns.

Per-core kernel (all matmuls fp32r, fp32 PSUM accumulate):
  inputs : xT [512, 2048]  (x[b] transposed, host-side)
           w  [512, 768]   (QKV projection columns for this head group)
  output : outT [256, 2048] (row h*64+d, col q) -- host transposes/interleaves.

  phase 1: Q^T, K^T = w_q/w_k^T-as-lhsT @ xT   -> [64*4, 2048] per-head rows
           V natural = xT-as-lhsT @ w_v        -> [2048, 4, 64] (+ ones col)
  phase 2: per (head, q-chunk of 1024), stream k-chunks of 128:
           dots^T[k, q] = K^T-chunk^T @ Q^T    (PSUM [128, 1024])
           P^T = exp(0.125 * dots^T)           (ScalarE, PSUM->SBUF, f32r)
           acc[65, q] += [V | 1]^T @ P^T       (row 64 = softmax denominator)
           out^T = acc[0:64] * bcast(1/acc[64])
"""
import sys

sys.path.insert(0, "/opt/trn_rl_repo")

import numpy as np

B, S, D = 4, 2048, 512
H_TOT, H, DH = 8, 4, 64  # heads total / per core
DC = D // 128  # 4 contraction chunks
KC = S // 128  # 16 k chunks
NQ = 4  # q chunks of QCH
QCH = 512
NJ = QCH // 512  # 512-wide matmul slices per q chunk
SCALE = DH ** -0.5

_CACHE = {}


def _build():
    import concourse.tile as tile
    from concourse import bacc, mybir

    F32 = mybir.dt.float32
    F32R = mybir.dt.float32r
    BF16 = mybir.dt.bfloat16
    EXP = mybir.ActivationFunctionType.Exp

    nc = bacc.Bacc("TRN2", target_bir_lowering=False, debug=False, num_devices=8)

    xT_d = nc.dram_tensor("xT", [D, S], F32, kind="ExternalInput")
    w_d = nc.dram_tensor("w", [D, 3 * H * DH], F32, kind="ExternalInput")
    out_d = nc.dram_tensor("outT", [H * DH, S], F32, kind="ExternalOutput")

    with tile.TileContext(nc) as tc:
        with tc.tile_pool(name="resident", bufs=1) as res:
            xT = res.tile([128, DC, S], F32R)
            w = res.tile([128, DC, 3 * H * DH], F32R)
            qT = res.tile([128, 2, S], F32R)
            kT = res.tile([128, 2, S], F32R)
            v = res.tile([128, KC, H, DH + 1], BF16)
            ones = res.tile([65, 64], F32R)
            nc.vector.memset(ones[64:65, :].bitcast(F32), 1.0)

            # preload the exp table set early (hides the ~2.7us ACT_TABLE_LOAD
            # under the initial DMAs instead of stalling PE mid-kernel)
            pre_in = res.tile([1, 2], F32)
            pre_out = res.tile([1, 2], F32)
            nc.vector.memset(pre_in[:], 0.0)
            nc.scalar.activation(
                pre_out[:], pre_in[:], mybir.ActivationFunctionType.Exp, scale=1.0
            )

            # w first (small, unblocks lhsT); xT interleaved by s-slice so
            # projection groups can start as soon as their slice has landed.
            for dc in range(DC):
                nc.sync.dma_start(
                    w[:, dc, :], w_d[dc * 128 : (dc + 1) * 128, :].bitcast(F32R)
                )
            for sb in range(4):
                ss = slice(sb * 512, (sb + 1) * 512)
                for dc in range(DC):
                    nc.sync.dma_start(
                        xT[:, dc, ss],
                        xT_d[dc * 128 : (dc + 1) * 128, ss].bitcast(F32R),
                    )
            nc.vector.memset(v[:, :, :, DH : DH + 1], 1.0)

            # ---- phase 1: projections ----
            with tc.tile_pool(name="proj_ps", bufs=4, space="PSUM") as proj_ps:
                # Q^T / K^T: lhsT = w columns (4 chunks of 128: q01,q23,k01,k23)
                for cc in range(4):
                    for sc in range(4):
                        ps = proj_ps.tile([128, 512], F32)
                        for dc in range(DC):
                            nc.tensor.matmul(
                                ps[:],
                                w[:, dc, cc * 128 : (cc + 1) * 128],
                                xT[:, dc, sc * 512 : (sc + 1) * 512],
                                start=(dc == 0),
                                stop=(dc == DC - 1),
                            )
                        dst = qT if cc < 2 else kT
                        nc.vector.tensor_copy(
                            dst[:, cc % 2, sc * 512 : (sc + 1) * 512], ps[:]
                        )
                # V natural: lhsT = xT chunks, rhs = w_v columns
                for sc in range(KC):
                    ps = proj_ps.tile([128, 256], F32, tag="vps")
                    for dc in range(DC):
                        nc.tensor.matmul(
                            ps[:],
                            xT[:, dc, sc * 128 : (sc + 1) * 128],
                            w[:, dc, 2 * H * DH : 3 * H * DH],
                            start=(dc == 0),
                            stop=(dc == DC - 1),
                        )
                    nc.vector.tensor_copy(
                        v[:, sc, :, 0:DH],
                        ps[:].rearrange("p (h d) -> p h d", h=H),
                    )

            # ---- phase 2: attention ----
            # Combined two-head dots tile per k-chunk: one FD-1024 exp per kc
            # amortizes the ScalarE per-instruction bubble. Dummy bf16 matmuls
            # (DUMMY_PER_KC) keep TensorE duty high enough that the clock
            # governor holds K=8/8 (2.4 GHz) through the exp-gated phase.
            DUMMY_PER_KC = 1
            with (
                tc.tile_pool(name="dots_ps", bufs=2, space="PSUM") as dots_ps,
                tc.tile_pool(name="oacc_ps", bufs=4, space="PSUM") as oacc_ps,
                tc.tile_pool(name="pt", bufs=3) as pt_pool,
                tc.tile_pool(name="norm", bufs=2) as norm_pool,
                tc.tile_pool(name="outp", bufs=2) as out_pool,
            ):
                for hc in range(2):
                    for qc in range(NQ):
                        q0 = qc * QCH
                        oaccs = [
                            oacc_ps.tile(
                                [DH + 1, QCH], F32, tag="oacc", name=f"oacc{_hp}"
                            )
                            for _hp in range(2)
                        ]
                        for kc in range(KC):
                            dots = dots_ps.tile([128, 2, QCH], F32, tag="dots")
                            for hp in range(2):
                                pb = hp * 64
                                nc.tensor.matmul(
                                    dots[:, hp, :],
                                    kT[pb : pb + 64, hc, kc * 128 : (kc + 1) * 128],
                                    qT[pb : pb + 64, hc, q0 : q0 + QCH],
                                    start=True,
                                    stop=True,
                                )
                            pT = pt_pool.tile([128, 2, QCH], BF16, tag="pt")
                            nc.scalar.activation(pT[:], dots[:], EXP, scale=SCALE)
                            for hp in range(2):
                                h = 2 * hc + hp
                                nc.tensor.matmul(
                                    oaccs[hp][:],
                                    v[:, kc, h, :],
                                    pT[:, hp, :],
                                    start=(kc == 0),
                                    stop=(kc == KC - 1),
                                )
                            for du in range(DUMMY_PER_KC):
                                dummy = oacc_ps.tile(
                                    [DH + 1, QCH], F32, tag="oacc", name="dummy"
                                )
                                nc.tensor.matmul(
                                    dummy[:],
                                    pT[:, 0, 0 : DH + 1],
                                    pT[:, 1, :],
                                    start=True,
                                    stop=True,
                                    skip_group_check=True,
                                )
                        osbs = []
                        for hp in range(2):
                            osb = norm_pool.tile(
                                [DH + 1, QCH], F32, tag="osb", name=f"osb{hp}"
                            )
                            nc.vector.tensor_copy(osb[:], oaccs[hp][:])
                            osbs.append(osb)
                        for hp in range(2):
                            h = 2 * hc + hp
                            osb = osbs[hp]
                            # full-tile: custom DVE ops misbehave at nonzero
                            # base partition; rows 0:64 are unused garbage.
                            recf = norm_pool.tile([DH + 1, QCH], F32, tag="recf")
                            nc.vector.reciprocal_approx_fast(recf[:], osb[:])
                            recr = norm_pool.tile([DH + 1, QCH], F32R, tag="recr")
                            with nc.allow_low_precision(
                                reason="f32r recip of softmax denom (~1e-4)"
                            ):
                                nc.vector.tensor_copy(
                                    recr[DH : DH + 1, :], recf[DH : DH + 1, :]
                                )
                            # broadcast recip row to 64 partitions via K=1 matmul
                            rb_ps = oacc_ps.tile(
                                [DH + 1, QCH], F32, tag="oacc", name="rbps"
                            )
                            nc.tensor.matmul(
                                rb_ps[0:DH, :],
                                ones[64:65, :],
                                recr[DH : DH + 1, :],
                                start=True,
                                stop=True,
                            )
                            rb = norm_pool.tile([DH, QCH], F32, tag="rb")
                            nc.vector.tensor_copy(rb[:], rb_ps[0:DH, :])
                            out_t = out_pool.tile([DH, QCH], F32)
                            nc.vector.tensor_mul(out_t[:], osb[0:DH, :], rb[:])
                            nc.sync.dma_start(
                                out_d[h * DH : (h + 1) * DH, q0 : q0 + QCH],
                                out_t[:],
                            )

    nc.compile()
    return nc


def kernel(x, w_qkv):
    from concourse.bass_utils import run_bass_kernel_spmd

    x = np.asarray(x)
    w_qkv = np.asarray(w_qkv)

    if "nc" not in _CACHE:
        _CACHE["nc"] = _build()
    nc = _CACHE["nc"]

    in_maps = []
    for i in range(8):
        b, hg = i // 2, i % 2
        xT = np.ascontiguousarray(x[b].T, dtype=np.float32)
        cols = slice(hg * H * DH, (hg + 1) * H * DH)
        w_i = np.ascontiguousarray(
            np.concatenate(
                [
                    w_qkv[:, 0 * D :][:, cols],
                    w_qkv[:, 1 * D :][:, cols],
                    w_qkv[:, 2 * D :][:, cols],
                ],
                axis=1,
            ),
            dtype=np.float32,
        )
        in_maps.append({"xT": xT, "w": w_i})
    _CACHE["in_maps"] = in_maps

    res = run_bass_kernel_spmd(nc, in_maps, list(range(8)))

    out = np.empty((B, S, D), dtype=np.float32)
    for i in range(8):
        b, hg = i // 2, i % 2
        out[b, :, hg * H * DH : (hg + 1) * H * DH] = res.results[i]["outT"].T
    return out


# revision 19
# speedup vs baseline: 1.7168x; 1.0662x over previous
"""Multi-head attention (B=4, S=2048, D=512, H=8, Dh=64) on 8 Trainium2 cores.

Sharding: core i handles batch b = i//2 and head-group hg = i%2 (4 heads each).
Data parallel on B, tensor parallel on heads / QKV projection columns.

Per-core kernel (all matmuls fp32r, fp32 PSUM accumulate):
  inputs : xT [512, 2048]  (x[b] transposed, host-side)
           w  [512, 768]   (QKV projection columns for this head group)
  output : outT [256, 2048] (row h*64+d, col q) -- host transposes/interleaves.

  phase 1: Q^T, K^T = w_q/w_k^T-as-lhsT @ xT   -> [64*4, 2048] per-head rows
           V natural = xT-as-lhsT @ w_v        -> [2048, 4, 64] (+ ones col)
  phase 2: per (head, q-chunk of 1024), stream k-chunks of 128:
           dots^T[k, q] = K^T-chunk^T @ Q^T    (PSUM [128, 1024])
           P^T = exp(0.125 * dots^T)           (ScalarE, PSUM->SBUF, f32r)
           acc[65, q] += [V | 1]^T @ P^T       (row 64 = softmax denominator)
           out^T = acc[0:64] * bcast(1/acc[64])
"""
import sys

sys.path.insert(0, "/opt/trn_rl_repo")

import numpy as np

B, S, D = 4, 2048, 512
H_TOT, H, DH = 8, 4, 64  # heads total / per core
DC = D // 128  # 4 contraction chunks
KC = S // 128  # 16 k chunks
NQ = 4  # q chunks of QCH
QCH = 512
NJ = QCH // 512  # 512-wide matmul slices per q chunk
SCALE = DH ** -0.5

_CACHE = {}


def _build():
    import concourse.tile as tile
    from concourse import bacc, mybir

    F32 = mybir.dt.float32
    F32R = mybir.dt.float32r
    BF16 = mybir.dt.bfloat16
    EXP = mybir.ActivationFunctionType.Exp

    nc = bacc.Bacc("TRN2", target_bir_lowering=False, debug=False, num_devices=8)

    xT_d = nc.dram_tensor("xT", [D, S], F32, kind="ExternalInput")
    w_d = nc.dram_tensor("w", [D, 3 * H * DH], F32, kind="ExternalInput")
    out_d = nc.dram_tensor("outT", [H * DH, S], F32, kind="ExternalOutput")

    with tile.TileContext(nc) as tc:
        with tc.tile_pool(name="resident", bufs=1) as res:
            xT = res.tile([128, DC, S], F32R)
            w = res.tile([128, DC, 3 * H * DH], F32R)
            qT = res.tile([128, 2, S], F32R)
            kT = res.tile([128, 2, S], F32R)
            v = res.tile([128, KC, H, DH + 1], BF16)
            ones = res.tile([65, 64], F32R)
            nc.vector.memset(ones[64:65, :].bitcast(F32), 1.0)

            # preload the exp table set early (hides the ~2.7us ACT_TABLE_LOAD
            # under the initial DMAs instead of stalling PE mid-kernel)
            pre_in = res.tile([1, 2], F32)
            pre_out = res.tile([1, 2], F32)
            nc.vector.memset(pre_in[:], 0.0)
            nc.scalar.activation(
                pre_out[:], pre_in[:], mybir.ActivationFunctionType.Exp, scale=1.0
            )

            # w first (small, unblocks lhsT); xT interleaved by s-slice so
            # projection groups can start as soon as their slice has landed.
            for dc in range(DC):
                nc.sync.dma_start(
                    w[:, dc, :], w_d[dc * 128 : (dc + 1) * 128, :].bitcast(F32R)
                )
            for sb in range(4):
                ss = slice(sb * 512, (sb + 1) * 512)
                for dc in range(DC):
                    nc.sync.dma_start(
                        xT[:, dc, ss],
                        xT_d[dc * 128 : (dc + 1) * 128, ss].bitcast(F32R),
                    )
            nc.vector.memset(v[:, :, :, DH : DH + 1], 1.0)

            # ---- phase 1: projections ----
            with tc.tile_pool(name="proj_ps", bufs=4, space="PSUM") as proj_ps:
                # Q^T / K^T: lhsT = w columns (4 chunks of 128: q01,q23,k01,k23)
                for cc in range(4):
                    for sc in range(4):
                        ps = proj_ps.tile([128, 512], F32)
                        for dc in range(DC):
                            nc.tensor.matmul(
                                ps[:],
                                w[:, dc, cc * 128 : (cc + 1) * 128],
                                xT[:, dc, sc * 512 : (sc + 1) * 512],
                                start=(dc == 0),
                                stop=(dc == DC - 1),
                            )
                        dst = qT if cc < 2 else kT
                        nc.vector.tensor_copy(
                            dst[:, cc % 2, sc * 512 : (sc + 1) * 512], ps[:]
                        )
                # V natural: lhsT = xT chunks, rhs = w_v columns
                for sc in range(KC):
                    ps = proj_ps.tile([128, 256], F32, tag="vps")
                    for dc in range(DC):
                        nc.tensor.matmul(
                            ps[:],
                            xT[:, dc, sc * 128 : (sc + 1) * 128],
                            w[:, dc, 2 * H * DH : 3 * H * DH],
                            start=(dc == 0),
                            stop=(dc == DC - 1),
                        )
                    nc.vector.tensor_copy(
                        v[:, sc, :, 0:DH],
                        ps[:].rearrange("p (h d) -> p h d", h=H),
                    )

            # ---- phase 2: attention ----
            # Combined two-head dots tile per k-chunk: one FD-1024 exp per kc
            # amortizes the ScalarE per-instruction bubble. Dummy bf16 matmuls
            # (DUMMY_PER_KC) keep TensorE duty high enough that the clock
            # governor holds K=8/8 (2.4 GHz) through the exp-gated phase.
            DUMMY_PER_KC = 1
            with (
                tc.tile_pool(name="dots_ps", bufs=2, space="PSUM") as dots_ps,
                tc.tile_pool(name="oacc_ps", bufs=4, space="PSUM") as oacc_ps,
                tc.tile_pool(name="pt", bufs=3) as pt_pool,
                tc.tile_pool(name="norm", bufs=2) as norm_pool,
                tc.tile_pool(name="outp", bufs=2) as out_pool,
            ):
                pending = []

                def flush_pending():
                    for fn in pending:
                        fn()
                    pending.clear()

                for hc in range(2):
                    for qc in range(NQ):
                        q0 = qc * QCH
                        oaccs = [
                            oacc_ps.tile(
                                [DH + 1, QCH], F32, tag="oacc", name=f"oacc{_hp}"
                            )
                            for _hp in range(2)
                        ]
                        for kc in range(KC):
                            dots = dots_ps.tile([128, 2, QCH], F32, tag="dots")
                            for hp in range(2):
                                pb = hp * 64
                                nc.tensor.matmul(
                                    dots[:, hp, :],
                                    kT[pb : pb + 64, hc, kc * 128 : (kc + 1) * 128],
                                    qT[pb : pb + 64, hc, q0 : q0 + QCH],
                                    start=True,
                                    stop=True,
                                )
                            pT = pt_pool.tile([128, 2, QCH], BF16, tag="pt")
                            nc.scalar.activation(pT[:], dots[:], EXP, scale=SCALE)
                            for hp in range(2):
                                h = 2 * hc + hp
                                nc.tensor.matmul(
                                    oaccs[hp][:],
                                    v[:, kc, h, :],
                                    pT[:, hp, :],
                                    start=(kc == 0),
                                    stop=(kc == KC - 1),
                                )
                            for du in range(DUMMY_PER_KC):
                                dummy = oacc_ps.tile(
                                    [DH + 1, QCH], F32, tag="oacc", name="dummy"
                                )
                                nc.tensor.matmul(
                                    dummy[:, 0:384],
                                    pT[:, 0, 0 : DH + 1],
                                    pT[:, 1, 0:384],
                                    start=True,
                                    stop=True,
                                    skip_group_check=True,
                                )
                            if kc == 3:
                                flush_pending()
                        osbs = []
                        for hp in range(2):
                            osb = norm_pool.tile(
                                [DH + 1, QCH], F32, tag="osb", name=f"osb{hp}"
                            )
                            nc.vector.tensor_copy(osb[:], oaccs[hp][:])
                            osbs.append(osb)
                        for hp in range(2):
                            h = 2 * hc + hp
                            osb = osbs[hp]
                            # full-tile: custom DVE ops misbehave at nonzero
                            # base partition; rows 0:64 are unused garbage.
                            recf = norm_pool.tile([DH + 1, QCH], F32, tag="recf")
                            nc.vector.reciprocal_approx_fast(recf[:], osb[:])
                            recr = norm_pool.tile([DH + 1, QCH], F32R, tag="recr")
                            with nc.allow_low_precision(
                                reason="f32r recip of softmax denom (~1e-4)"
                            ):
                                nc.vector.tensor_copy(
                                    recr[DH : DH + 1, :], recf[DH : DH + 1, :]
                                )

                            def finish(h=h, q0=q0, osb=osb, recr=recr):
                                # broadcast recip row to 64 partitions via
                                # K=1 matmul (deferred into the next section's
                                # k-loop so the PE queue never stalls on the
                                # DVE reciprocal chain)
                                rb_ps = oacc_ps.tile(
                                    [DH + 1, QCH], F32, tag="oacc", name="rbps"
                                )
                                nc.tensor.matmul(
                                    rb_ps[0:DH, :],
                                    ones[64:65, :],
                                    recr[DH : DH + 1, :],
                                    start=True,
                                    stop=True,
                                )
                                rb = norm_pool.tile([DH, QCH], F32, tag="rb")
                                nc.vector.tensor_copy(rb[:], rb_ps[0:DH, :])
                                out_t = out_pool.tile([DH, QCH], F32)
                                nc.vector.tensor_mul(out_t[:], osb[0:DH, :], rb[:])
                                nc.sync.dma_start(
                                    out_d[h * DH : (h + 1) * DH, q0 : q0 + QCH],
                                    out_t[:],
                                )

                            pending.append(finish)
                flush_pending()

    nc.compile()
    return nc


def kernel(x, w_qkv):
    from concourse.bass_utils import run_bass_kernel_spmd

    x = np.asarray(x)
    w_qkv = np.asarray(w_qkv)

    if "nc" not in _CACHE:
        _CACHE["nc"] = _build()
    nc = _CACHE["nc"]

    in_maps = []
    for i in range(8):
        b, hg = i // 2, i % 2
        xT = np.ascontiguousarray(x[b].T, dtype=np.float32)
        cols = slice(hg * H * DH, (hg + 1) * H * DH)
        w_i = np.ascontiguousarray(
            np.concatenate(
                [
                    w_qkv[:, 0 * D :][:, cols],
                    w_qkv[:, 1 * D :][:, cols],
                    w_qkv[:, 2 * D :][:, cols],
                ],
                axis=1,
            ),
            dtype=np.float32,
        )
        in_maps.append({"xT": xT, "w": w_i})
    _CACHE["in_maps"] = in_maps

    res = run_bass_kernel_spmd(nc, in_maps, list(range(8)))

    out = np.empty((B, S, D), dtype=np.float32)
    for i in range(8):
        b, hg = i // 2, i % 2
        out[b, :, hg * H * DH : (hg + 1) * H * DH] = res.results[i]["outT"].T
    return out


# revision 21
# speedup vs baseline: 1.7983x; 1.0474x over previous
"""Multi-head attention (B=4, S=2048, D=512, H=8, Dh=64) on 8 Trainium2 cores.

Sharding: core i handles batch b = i//2 and head-group hg = i%2 (4 heads each).
Data parallel on B, tensor parallel on heads / QKV projection columns.

Per-core kernel (all matmuls fp32r, fp32 PSUM accumulate):
  inputs : xT [512, 2048]  (x[b] transposed, host-side)
           w  [512, 768]   (QKV projection columns for this head group)
  output : outT [256, 2048] (row h*64+d, col q) -- host transposes/interleaves.

  phase 1: Q^T, K^T = w_q/w_k^T-as-lhsT @ xT   -> [64*4, 2048] per-head rows
           V natural = xT-as-lhsT @ w_v        -> [2048, 4, 64] (+ ones col)
  phase 2: per (head, q-chunk of 1024), stream k-chunks of 128:
           dots^T[k, q] = K^T-chunk^T @ Q^T    (PSUM [128, 1024])
           P^T = exp(0.125 * dots^T)           (ScalarE, PSUM->SBUF, f32r)
           acc[65, q] += [V | 1]^T @ P^T       (row 64 = softmax denominator)
           out^T = acc[0:64] * bcast(1/acc[64])
"""
import sys

sys.path.insert(0, "/opt/trn_rl_repo")

import numpy as np

B, S, D = 4, 2048, 512
H_TOT, H, DH = 8, 4, 64  # heads total / per core
DC = D // 128  # 4 contraction chunks
KC = S // 128  # 16 k chunks
NQ = 4  # q chunks of QCH
QCH = 512
NJ = QCH // 512  # 512-wide matmul slices per q chunk
SCALE = DH ** -0.5

_CACHE = {}


def _build():
    import concourse.tile as tile
    from concourse import bacc, mybir

    F32 = mybir.dt.float32
    F32R = mybir.dt.float32r
    BF16 = mybir.dt.bfloat16
    EXP = mybir.ActivationFunctionType.Exp

    nc = bacc.Bacc("TRN2", target_bir_lowering=False, debug=False, num_devices=8)

    xT_d = nc.dram_tensor("xT", [D, S], F32, kind="ExternalInput")
    w_d = nc.dram_tensor("w", [D, 3 * H * DH], F32, kind="ExternalInput")
    out_d = nc.dram_tensor("outT", [H * DH, S], F32, kind="ExternalOutput")

    with tile.TileContext(nc) as tc:
        with tc.tile_pool(name="resident", bufs=1) as res:
            xT = res.tile([128, DC, S], F32R)
            w = res.tile([128, DC, 3 * H * DH], F32R)
            qT = res.tile([128, 2, S], F32R)
            kT = res.tile([128, 2, S], F32R)
            v = res.tile([128, KC, H, DH + 1], BF16)
            ones = res.tile([65, 64], F32R)
            nc.vector.memset(ones[64:65, :].bitcast(F32), 1.0)

            # preload the exp table set early (hides the ~2.7us ACT_TABLE_LOAD
            # under the initial DMAs instead of stalling PE mid-kernel)
            pre_in = res.tile([1, 2], F32)
            pre_out = res.tile([1, 2], F32)
            nc.vector.memset(pre_in[:], 0.0)
            nc.scalar.activation(
                pre_out[:], pre_in[:], mybir.ActivationFunctionType.Exp, scale=1.0
            )

            # w first (small, unblocks lhsT); xT interleaved by s-slice so
            # projection groups can start as soon as their slice has landed.
            for dc in range(DC):
                nc.sync.dma_start(
                    w[:, dc, :], w_d[dc * 128 : (dc + 1) * 128, :].bitcast(F32R)
                )
            for sb in range(4):
                ss = slice(sb * 512, (sb + 1) * 512)
                for dc in range(DC):
                    nc.sync.dma_start(
                        xT[:, dc, ss],
                        xT_d[dc * 128 : (dc + 1) * 128, ss].bitcast(F32R),
                    )
            nc.vector.memset(v[:, :, :, DH : DH + 1], 1.0)

            # ---- phase 1: projections ----
            with tc.tile_pool(name="proj_ps", bufs=4, space="PSUM") as proj_ps:
                # Q^T / K^T: lhsT = w columns (4 chunks of 128: q01,q23,k01,k23)
                for cc in range(4):
                    for sc in range(4):
                        ps = proj_ps.tile([128, 512], F32)
                        for dc in range(DC):
                            nc.tensor.matmul(
                                ps[:],
                                w[:, dc, cc * 128 : (cc + 1) * 128],
                                xT[:, dc, sc * 512 : (sc + 1) * 512],
                                start=(dc == 0),
                                stop=(dc == DC - 1),
                            )
                        dst = qT if cc < 2 else kT
                        nc.vector.tensor_copy(
                            dst[:, cc % 2, sc * 512 : (sc + 1) * 512], ps[:]
                        )
                # V natural: lhsT = xT chunks, rhs = w_v columns
                for sc in range(KC):
                    ps = proj_ps.tile([128, 256], F32, tag="vps")
                    for dc in range(DC):
                        nc.tensor.matmul(
                            ps[:],
                            xT[:, dc, sc * 128 : (sc + 1) * 128],
                            w[:, dc, 2 * H * DH : 3 * H * DH],
                            start=(dc == 0),
                            stop=(dc == DC - 1),
                        )
                    nc.vector.tensor_copy(
                        v[:, sc, :, 0:DH],
                        ps[:].rearrange("p (h d) -> p h d", h=H),
                    )

            # ---- phase 2: attention ----
            # Combined two-head dots tile per k-chunk: one FD-1024 exp per kc
            # amortizes the ScalarE per-instruction bubble. Dummy bf16 matmuls
            # (DUMMY_PER_KC) keep TensorE duty high enough that the clock
            # governor holds K=8/8 (2.4 GHz) through the exp-gated phase.
            DUMMY_PER_KC = 1
            with (
                tc.tile_pool(name="dots_ps", bufs=2, space="PSUM") as dots_ps,
                tc.tile_pool(name="oacc_ps", bufs=4, space="PSUM") as oacc_ps,
                tc.tile_pool(name="pt", bufs=3) as pt_pool,
                tc.tile_pool(name="norm", bufs=2) as norm_pool,
                tc.tile_pool(name="outp", bufs=2) as out_pool,
                tc.tile_pool(name="dscr", bufs=2, space="DRAM") as dram_pool,
            ):
                pending = []

                def flush_pending():
                    for fn in pending:
                        fn()
                    pending.clear()

                for hc in range(2):
                    for qc in range(NQ):
                        q0 = qc * QCH
                        oaccs = [
                            oacc_ps.tile(
                                [DH + 1, QCH], F32, tag="oacc", name=f"oacc{_hp}"
                            )
                            for _hp in range(2)
                        ]
                        for kc in range(KC):
                            dots = dots_ps.tile([128, 2, QCH], F32, tag="dots")
                            for hp in range(2):
                                pb = hp * 64
                                nc.tensor.matmul(
                                    dots[:, hp, :],
                                    kT[pb : pb + 64, hc, kc * 128 : (kc + 1) * 128],
                                    qT[pb : pb + 64, hc, q0 : q0 + QCH],
                                    start=True,
                                    stop=True,
                                )
                            pT = pt_pool.tile([128, 2, QCH], BF16, tag="pt")
                            nc.scalar.activation(pT[:], dots[:], EXP, scale=SCALE)
                            for hp in range(2):
                                h = 2 * hc + hp
                                nc.tensor.matmul(
                                    oaccs[hp][:],
                                    v[:, kc, h, :],
                                    pT[:, hp, :],
                                    start=(kc == 0),
                                    stop=(kc == KC - 1),
                                )
                            for du in range(DUMMY_PER_KC):
                                dummy = oacc_ps.tile(
                                    [DH + 1, QCH], F32, tag="oacc", name="dummy"
                                )
                                nc.tensor.matmul(
                                    dummy[:, 0:384],
                                    pT[:, 0, 0 : DH + 1],
                                    pT[:, 1, 0:384],
                                    start=True,
                                    stop=True,
                                    skip_group_check=True,
                                )
                            if kc == 3:
                                flush_pending()
                        osbs = []
                        for hp in range(2):
                            osb = norm_pool.tile(
                                [DH + 1, QCH], F32, tag="osb", name=f"osb{hp}"
                            )
                            nc.vector.tensor_copy(osb[:], oaccs[hp][:])
                            osbs.append(osb)
                        for hp in range(2):
                            h = 2 * hc + hp
                            osb = osbs[hp]
                            # full-tile: custom DVE ops misbehave at nonzero
                            # base partition; rows 0:64 are unused garbage.
                            recf = norm_pool.tile([DH + 1, QCH], F32, tag="recf")
                            nc.vector.reciprocal_approx_fast(recf[:], osb[:])

                            def finish(h=h, q0=q0, osb=osb, recf=recf):
                                # broadcast recip row to 64 partitions: bounce
                                # via a DRAM row, then a partition-step-0 DMA
                                # read back (deferred into the next section's
                                # k-loop; no PE work)
                                import concourse.bass as _bass

                                drow = dram_pool.tile([1, QCH], F32, name="drow")
                                nc.sync.dma_start(drow[:], recf[DH : DH + 1, :])
                                bcast = _bass.AP(
                                    tensor=drow.tensor,
                                    offset=drow.offset,
                                    ap=[[0, DH]] + list(drow.ap[1:]),
                                )
                                rb = norm_pool.tile([DH, QCH], F32, tag="rb")
                                nc.sync.dma_start(rb[:], bcast)
                                out_t = out_pool.tile([DH, QCH], F32)
                                nc.vector.tensor_mul(out_t[:], osb[0:DH, :], rb[:])
                                nc.sync.dma_start(
                                    out_d[h * DH : (h + 1) * DH, q0 : q0 + QCH],
                                    out_t[:],
                                )

                            pending.append(finish)
                flush_pending()

    nc.compile()
    return nc


def kernel(x, w_qkv):
    from concourse.bass_utils import run_bass_kernel_spmd

    x = np.asarray(x)
    w_qkv = np.asarray(w_qkv)

    if "nc" not in _CACHE:
        _CACHE["nc"] = _build()
    nc = _CACHE["nc"]

    in_maps = []
    for i in range(8):
        b, hg = i // 2, i % 2
        xT = np.ascontiguousarray(x[b].T, dtype=np.float32)
        cols = slice(hg * H * DH, (hg + 1) * H * DH)
        w_i = np.ascontiguousarray(
            np.concatenate(
                [
                    w_qkv[:, 0 * D :][:, cols],
                    w_qkv[:, 1 * D :][:, cols],
                    w_qkv[:, 2 * D :][:, cols],
                ],
                axis=1,
            ),
            dtype=np.float32,
        )
        in_maps.append({"xT": xT, "w": w_i})
    _CACHE["in_maps"] = in_maps

    res = run_bass_kernel_spmd(nc, in_maps, list(range(8)))

    out = np.empty((B, S, D), dtype=np.float32)
    for i in range(8):
        b, hg = i // 2, i % 2
        out[b, :, hg * H * DH : (hg + 1) * H * DH] = res.results[i]["outT"].T
    return out


# revision 22
# speedup vs baseline: 1.8843x; 1.0478x over previous
"""Multi-head attention (B=4, S=2048, D=512, H=8, Dh=64) on 8 Trainium2 cores.

Sharding: core i handles batch b = i//2 and head-group hg = i%2 (4 heads each).
Data parallel on B, tensor parallel on heads / QKV projection columns.

Per-core kernel (all matmuls fp32r, fp32 PSUM accumulate):
  inputs : xT [512, 2048]  (x[b] transposed, host-side)
           w  [512, 768]   (QKV projection columns for this head group)
  output : outT [256, 2048] (row h*64+d, col q) -- host transposes/interleaves.

  phase 1: Q^T, K^T = w_q/w_k^T-as-lhsT @ xT   -> [64*4, 2048] per-head rows
           V natural = xT-as-lhsT @ w_v        -> [2048, 4, 64] (+ ones col)
  phase 2: per (head, q-chunk of 1024), stream k-chunks of 128:
           dots^T[k, q] = K^T-chunk^T @ Q^T    (PSUM [128, 1024])
           P^T = exp(0.125 * dots^T)           (ScalarE, PSUM->SBUF, f32r)
           acc[65, q] += [V | 1]^T @ P^T       (row 64 = softmax denominator)
           out^T = acc[0:64] * bcast(1/acc[64])
"""
import sys

sys.path.insert(0, "/opt/trn_rl_repo")

import numpy as np

B, S, D = 4, 2048, 512
H_TOT, H, DH = 8, 4, 64  # heads total / per core
DC = D // 128  # 4 contraction chunks
KC = S // 128  # 16 k chunks
NQ = 4  # q chunks of QCH
QCH = 512
NJ = QCH // 512  # 512-wide matmul slices per q chunk
SCALE = DH ** -0.5

_CACHE = {}


def _build():
    import concourse.tile as tile
    from concourse import bacc, mybir

    F32 = mybir.dt.float32
    F32R = mybir.dt.float32r
    BF16 = mybir.dt.bfloat16
    EXP = mybir.ActivationFunctionType.Exp

    nc = bacc.Bacc("TRN2", target_bir_lowering=False, debug=False, num_devices=8)

    xT_d = nc.dram_tensor("xT", [D, S], F32, kind="ExternalInput")
    w_d = nc.dram_tensor("w", [D, 3 * H * DH], F32, kind="ExternalInput")
    out_d = nc.dram_tensor("outT", [H * DH, S], F32, kind="ExternalOutput")

    with tile.TileContext(nc) as tc:
        with tc.tile_pool(name="resident", bufs=1) as res:
            xT = res.tile([128, DC, S], F32R)
            w = res.tile([128, DC, 3 * H * DH], F32R)
            qT = res.tile([128, 2, S], F32R)
            kT = res.tile([128, 2, S], F32R)
            v = res.tile([128, KC, H, DH + 1], BF16)
            ones = res.tile([65, 64], F32R)
            nc.vector.memset(ones[64:65, :].bitcast(F32), 1.0)

            # preload the exp table set early (hides the ~2.7us ACT_TABLE_LOAD
            # under the initial DMAs instead of stalling PE mid-kernel)
            pre_in = res.tile([1, 2], F32)
            pre_out = res.tile([1, 2], F32)
            nc.vector.memset(pre_in[:], 0.0)
            nc.scalar.activation(
                pre_out[:], pre_in[:], mybir.ActivationFunctionType.Exp, scale=1.0
            )

            # w first (small, unblocks lhsT); xT interleaved by s-slice so
            # projection groups can start as soon as their slice has landed.
            for dc in range(DC):
                nc.sync.dma_start(
                    w[:, dc, :], w_d[dc * 128 : (dc + 1) * 128, :].bitcast(F32R)
                )
            for sb in range(4):
                ss = slice(sb * 512, (sb + 1) * 512)
                for dc in range(DC):
                    nc.sync.dma_start(
                        xT[:, dc, ss],
                        xT_d[dc * 128 : (dc + 1) * 128, ss].bitcast(F32R),
                    )
            nc.vector.memset(v[:, :, :, DH : DH + 1], 1.0)

            # ---- phase 1: projections ----
            with tc.tile_pool(name="proj_ps", bufs=4, space="PSUM") as proj_ps:
                # Q^T / K^T: lhsT = w columns (4 chunks of 128: q01,q23,k01,k23)
                for cc in range(4):
                    for sc in range(4):
                        ps = proj_ps.tile([128, 512], F32)
                        for dc in range(DC):
                            nc.tensor.matmul(
                                ps[:],
                                w[:, dc, cc * 128 : (cc + 1) * 128],
                                xT[:, dc, sc * 512 : (sc + 1) * 512],
                                start=(dc == 0),
                                stop=(dc == DC - 1),
                            )
                        dst = qT if cc < 2 else kT
                        nc.vector.tensor_copy(
                            dst[:, cc % 2, sc * 512 : (sc + 1) * 512], ps[:]
                        )
                # V natural: lhsT = xT chunks, rhs = w_v columns
                for sc in range(KC):
                    ps = proj_ps.tile([128, 256], F32, tag="vps")
                    for dc in range(DC):
                        nc.tensor.matmul(
                            ps[:],
                            xT[:, dc, sc * 128 : (sc + 1) * 128],
                            w[:, dc, 2 * H * DH : 3 * H * DH],
                            start=(dc == 0),
                            stop=(dc == DC - 1),
                        )
                    nc.vector.tensor_copy(
                        v[:, sc, :, 0:DH],
                        ps[:].rearrange("p (h d) -> p h d", h=H),
                    )

            # ---- phase 2: attention ----
            # Combined two-head dots tile per k-chunk: one FD-1024 exp per kc
            # amortizes the ScalarE per-instruction bubble. Dummy bf16 matmuls
            # (DUMMY_PER_KC) keep TensorE duty high enough that the clock
            # governor holds K=8/8 (2.4 GHz) through the exp-gated phase.
            DUMMY_PER_KC = 0
            with (
                tc.tile_pool(name="dots_ps", bufs=2, space="PSUM") as dots_ps,
                tc.tile_pool(name="oacc_ps", bufs=4, space="PSUM") as oacc_ps,
                tc.tile_pool(name="pt", bufs=3) as pt_pool,
                tc.tile_pool(name="norm", bufs=2) as norm_pool,
                tc.tile_pool(name="outp", bufs=2) as out_pool,
                tc.tile_pool(name="dscr", bufs=2, space="DRAM") as dram_pool,
            ):
                pending = []

                def flush_pending():
                    for fn in pending:
                        fn()
                    pending.clear()

                for hc in range(2):
                    for qc in range(NQ):
                        q0 = qc * QCH
                        oaccs = [
                            oacc_ps.tile(
                                [DH + 1, QCH], F32, tag="oacc", name=f"oacc{_hp}"
                            )
                            for _hp in range(2)
                        ]
                        for kc in range(KC):
                            dots = dots_ps.tile([128, 2, QCH], F32, tag="dots")
                            for hp in range(2):
                                pb = hp * 64
                                nc.tensor.matmul(
                                    dots[:, hp, :],
                                    kT[pb : pb + 64, hc, kc * 128 : (kc + 1) * 128],
                                    qT[pb : pb + 64, hc, q0 : q0 + QCH],
                                    start=True,
                                    stop=True,
                                )
                            pT = pt_pool.tile([128, 2, QCH], BF16, tag="pt")
                            nc.scalar.activation(pT[:], dots[:], EXP, scale=SCALE)
                            for hp in range(2):
                                h = 2 * hc + hp
                                nc.tensor.matmul(
                                    oaccs[hp][:],
                                    v[:, kc, h, :],
                                    pT[:, hp, :],
                                    start=(kc == 0),
                                    stop=(kc == KC - 1),
                                )
                            for du in range(DUMMY_PER_KC):
                                dummy = oacc_ps.tile(
                                    [DH + 1, QCH], F32, tag="oacc", name="dummy"
                                )
                                nc.tensor.matmul(
                                    dummy[:, 0:384],
                                    pT[:, 0, 0 : DH + 1],
                                    pT[:, 1, 0:384],
                                    start=True,
                                    stop=True,
                                    skip_group_check=True,
                                )
                            if kc == 3:
                                flush_pending()
                        osbs = []
                        for hp in range(2):
                            osb = norm_pool.tile(
                                [DH + 1, QCH], F32, tag="osb", name=f"osb{hp}"
                            )
                            nc.vector.tensor_copy(osb[:], oaccs[hp][:])
                            osbs.append(osb)
                        for hp in range(2):
                            h = 2 * hc + hp
                            osb = osbs[hp]
                            # full-tile: custom DVE ops misbehave at nonzero
                            # base partition; rows 0:64 are unused garbage.
                            recf = norm_pool.tile([DH + 1, QCH], F32, tag="recf")
                            nc.vector.reciprocal_approx_fast(recf[:], osb[:])

                            def finish(h=h, q0=q0, osb=osb, recf=recf):
                                # broadcast recip row to 64 partitions: bounce
                                # via a DRAM row, then a partition-step-0 DMA
                                # read back (deferred into the next section's
                                # k-loop; no PE work)
                                import concourse.bass as _bass

                                drow = dram_pool.tile([1, QCH], F32, name="drow")
                                nc.sync.dma_start(drow[:], recf[DH : DH + 1, :])
                                bcast = _bass.AP(
                                    tensor=drow.tensor,
                                    offset=drow.offset,
                                    ap=[[0, DH]] + list(drow.ap[1:]),
                                )
                                rb = norm_pool.tile([DH, QCH], F32, tag="rb")
                                nc.sync.dma_start(rb[:], bcast)
                                out_t = out_pool.tile([DH, QCH], F32)
                                nc.vector.tensor_mul(out_t[:], osb[0:DH, :], rb[:])
                                nc.sync.dma_start(
                                    out_d[h * DH : (h + 1) * DH, q0 : q0 + QCH],
                                    out_t[:],
                                )

                            pending.append(finish)
                flush_pending()

    nc.compile()
    return nc


def kernel(x, w_qkv):
    from concourse.bass_utils import run_bass_kernel_spmd

    x = np.asarray(x)
    w_qkv = np.asarray(w_qkv)

    if "nc" not in _CACHE:
        _CACHE["nc"] = _build()
    nc = _CACHE["nc"]

    in_maps = []
    for i in range(8):
        b, hg = i // 2, i % 2
        xT = np.ascontiguousarray(x[b].T, dtype=np.float32)
        cols = slice(hg * H * DH, (hg + 1) * H * DH)
        w_i = np.ascontiguousarray(
            np.concatenate(
                [
                    w_qkv[:, 0 * D :][:, cols],
                    w_qkv[:, 1 * D :][:, cols],
                    w_qkv[:, 2 * D :][:, cols],
                ],
                axis=1,
            ),
            dtype=np.float32,
        )
        in_maps.append({"xT": xT, "w": w_i})
    _CACHE["in_maps"] = in_maps

    res = run_bass_kernel_spmd(nc, in_maps, list(range(8)))

    out = np.empty((B, S, D), dtype=np.float32)
    for i in range(8):
        b, hg = i // 2, i % 2
        out[b, :, hg * H * DH : (hg + 1) * H * DH] = res.results[i]["outT"].T
    return out
